# revision 4
# baseline (speedup 1.0000x reference)
"""Bahdanau additive-attention decoder kernel for one TRN2 chip (8 NeuronCores).

Reference computation (B=4, L=128, C=1024, D=512):
    wq = hidden @ W.T                      # [B, L, D]
    uc = ctx @ U.T + U_b                   # [B, C, D]
    align[b,l,c] = sum_d V[d] * tanh(wq[b,l,d] + uc[b,c,d])
    out = softmax(align, axis=-1)          # [B, L, C]

Sharding: pure data-parallel over the (B x L) grid -- core i owns batch
b = i//2 and L-half h = i%2 (64 query rows), with the full ctx[b].  No
collectives are needed; softmax is over the unsharded C axis.

Per-core device algorithm (layout: d on partitions, c on free axis):
    ucT[d, c] = sum_k U.T[k, d-chunk] . ctxT[k, c]      (PE, fp32)
    wqT[d, l] = sum_k W.T[k, d-chunk] . hiddenT[k, l] + U_b[d]
    for each l:  s = tanh(ucT + wqT[:, l])  via ACT bias-fused tanh (bf16 out)
                 align[l, :] += V_chunk.T @ s  (PE matvec, M=1, accum over 4 chunks)
    softmax over free axis, DMA out.

The host pre-transposes the operands (layout prep is part of sharding);
all arithmetic runs on device.
"""

import os

import numpy as np
import ml_dtypes

import concourse.bass as bass
import concourse.mybir as mybir
import concourse.tile as tile
from concourse import bacc
from concourse.bass_utils import run_bass_kernel_spmd

B, L, C, D = 4, 128, 1024, 512
N_CORES = 8
LSH = (B * L) // N_CORES  # 64 query rows per core
KCH = D // 128  # 4 contraction chunks of 128
F32 = mybir.dt.float32
BF16 = mybir.dt.bfloat16

_CACHE = {}


def _install_ntff_hook_shim():
    """The agent image's antenv lacks axon_hooks; provide it so
    run_bass_kernel_spmd(trace=True) can capture NTFF profiles through
    the axon .so (same mechanism trn_boot would have installed)."""
    import sys
    import types

    try:
        from antenv.axon_hooks import get_axon_ntff_profile_hook

        if get_axon_ntff_profile_hook() is not None:
            return
    except ImportError:
        mod = types.ModuleType("antenv.axon_hooks")
        mod._hook = None

        def set_axon_ntff_profile_hook(h):
            mod._hook = h

        def get_axon_ntff_profile_hook():
            return mod._hook

        mod.set_axon_ntff_profile_hook = set_axon_ntff_profile_hook
        mod.get_axon_ntff_profile_hook = get_axon_ntff_profile_hook
        sys.modules["antenv.axon_hooks"] = mod
        import antenv

        antenv.axon_hooks = mod

    from trn_agent_boot.trn_boot import _ntff_profile_via_ctypes
    import antenv.axon_hooks as ah

    for so in ("/opt/axon/libaxon_pjrt.so",):
        if os.path.exists(so):
            hook = _ntff_profile_via_ctypes(so)
            if hook is not None:
                ah.set_axon_ntff_profile_hook(hook)
                return


def _build():
    nc = bacc.Bacc(
        "TRN2",
        target_bir_lowering=False,
        debug=False,
        num_devices=N_CORES,
    )

    ctxT = nc.dram_tensor("ctxT", (D, C), F32, kind="ExternalInput").ap()
    hidT = nc.dram_tensor("hidT", (D, LSH), F32, kind="ExternalInput").ap()
    UT = nc.dram_tensor("UT", (D, D), F32, kind="ExternalInput").ap()
    WT = nc.dram_tensor("WT", (D, D), F32, kind="ExternalInput").ap()
    Ub = nc.dram_tensor("Ub", (D, 1), F32, kind="ExternalInput").ap()
    Vb = nc.dram_tensor("Vb", (D, 1), BF16, kind="ExternalInput").ap()
    out = nc.dram_tensor("out", (LSH, C), F32, kind="ExternalOutput").ap()

    with tile.TileContext(nc) as tc:
        with (
            tc.tile_pool(name="consts", bufs=1) as cp,
            tc.tile_pool(name="s_pool", bufs=2) as sp,
            tc.tile_pool(name="uc_ps", bufs=2, space="PSUM") as ucp,
            tc.tile_pool(name="al_ps", bufs=2, space="PSUM") as alp,
        ):
            # ---- stage inputs in SBUF ----
            ctxT_t = []
            UT_t = []
            WT_t = []
            hidT_t = []
            ub_t = []
            vb_t = []
            for k in range(KCH):
                sl = slice(k * 128, (k + 1) * 128)
                t = cp.tile([128, C], F32, name=f"ctxT{k}", tag=f"ctxT{k}")
                nc.sync.dma_start(out=t, in_=ctxT[sl, :])
                ctxT_t.append(t)
                t = cp.tile([128, D], F32, name=f"UT{k}", tag=f"UT{k}")
                nc.sync.dma_start(out=t, in_=UT[sl, :])
                UT_t.append(t)
                t = cp.tile([128, D], F32, name=f"WT{k}", tag=f"WT{k}")
                nc.sync.dma_start(out=t, in_=WT[sl, :])
                WT_t.append(t)
                t = cp.tile([128, LSH], F32, name=f"hidT{k}", tag=f"hidT{k}")
                nc.sync.dma_start(out=t, in_=hidT[sl, :])
                hidT_t.append(t)
                t = cp.tile([128, 1], F32, name=f"ub{k}", tag=f"ub{k}")
                nc.sync.dma_start(out=t, in_=Ub[sl, :])
                ub_t.append(t)
                t = cp.tile([128, 1], BF16, name=f"vb{k}", tag=f"vb{k}")
                nc.sync.dma_start(out=t, in_=Vb[sl, :])
                vb_t.append(t)

            # ---- ucT[e_chunk] = (ctx @ U.T).T in [d-part, c-free] layout ----
            ucT_t = []
            for e in range(KCH):
                esl = slice(e * 128, (e + 1) * 128)
                uct = cp.tile([128, C], F32, name=f"ucT{e}", tag=f"ucT{e}")
                for half in range(2):
                    hsl = slice(half * 512, (half + 1) * 512)
                    ps = ucp.tile([128, 512], F32, name=f"ucps{e}_{half}", tag="ucps")
                    for k in range(KCH):
                        nc.tensor.matmul(
                            ps,
                            lhsT=UT_t[k][:, esl],
                            rhs=ctxT_t[k][:, hsl],
                            start=(k == 0),
                            stop=(k == KCH - 1),
                        )
                    nc.vector.tensor_copy(out=uct[:, hsl], in_=ps)
                ucT_t.append(uct)

            # ---- wqT[e_chunk] = (hidden @ W.T).T + U_b, [d-part, l-free] ----
            wqT_t = []
            for e in range(KCH):
                esl = slice(e * 128, (e + 1) * 128)
                ps = ucp.tile([128, LSH], F32, name=f"wqps{e}", tag="wqps")
                for k in range(KCH):
                    nc.tensor.matmul(
                        ps,
                        lhsT=WT_t[k][:, esl],
                        rhs=hidT_t[k],
                        start=(k == 0),
                        stop=(k == KCH - 1),
                    )
                wqt = cp.tile([128, LSH], F32, name=f"wqT{e}", tag=f"wqT{e}")
                nc.vector.tensor_scalar_add(out=wqt, in0=ps, scalar1=ub_t[e])
                wqT_t.append(wqt)

            align_t = cp.tile([LSH, C], F32, name="align", tag="align")

            # ---- main loop: tanh (ACT, bias-fused) + V matvec (PE) ----
            # PE emits align rows as [1, C] on PSUM partition 0; compute
            # engines cannot shift partitions, so rows are staged on
            # partition 0 and scattered into align_t rows by SBUF->SBUF DMA
            # in groups of GRP.
            GRP = 8
            for g in range(LSH // GRP):
                stage = sp.tile([1, GRP, C], F32, name=f"stage{g}", tag="stage")
                for j in range(GRP):
                    l = g * GRP + j
                    s_tiles = []
                    for k in range(KCH):
                        st = sp.tile([128, C], BF16, name=f"s{k}_{l}", tag=f"s{k}")
                        nc.scalar.activation(
                            out=st,
                            in_=ucT_t[k],
                            func=mybir.ActivationFunctionType.Tanh,
                            bias=wqT_t[k][:, l : l + 1],
                            scale=1.0,
                        )
                        s_tiles.append(st)
                    ps = alp.tile([1, C], F32, name=f"alps{l}", tag="al")
                    for half in range(2):
                        hsl = slice(half * 512, (half + 1) * 512)
                        for k in range(KCH):
                            nc.tensor.matmul(
                                ps[0:1, hsl],
                                lhsT=vb_t[k],
                                rhs=s_tiles[k][:, hsl],
                                start=(k == 0),
                                stop=(k == KCH - 1),
                            )
                    nc.vector.tensor_copy(out=stage[0:1, j, :], in_=ps)
                nc.sync.dma_start(
                    out=align_t[g * GRP : (g + 1) * GRP, :], in_=stage[0:1, :, :]
                )

            # ---- softmax over free axis (c) ----
            negmax = cp.tile([LSH, 1], F32, name="negmax", tag="negmax")
            nc.vector.tensor_reduce(
                out=negmax,
                in_=align_t,
                axis=mybir.AxisListType.X,
                op=mybir.AluOpType.max,
                negate=True,
            )
            esum = cp.tile([LSH, 1], F32, name="esum", tag="esum")
            nc.scalar.activation(
                out=align_t,
                in_=align_t,
                func=mybir.ActivationFunctionType.Exp,
                bias=negmax,
                scale=1.0,
                accum_out=esum,
            )
            rec = cp.tile([LSH, 1], F32, name="rec", tag="rec")
            nc.vector.reciprocal(out=rec, in_=esum)
            nc.vector.tensor_scalar_mul(out=align_t, in0=align_t, scalar1=rec)
            nc.sync.dma_start(out=out, in_=align_t)

    nc.compile()
    return nc


def kernel(hidden, ctx, W, U, U_b, V):
    hidden = np.asarray(hidden, dtype=np.float32)
    ctx = np.asarray(ctx, dtype=np.float32)
    W = np.asarray(W, dtype=np.float32)
    U = np.asarray(U, dtype=np.float32)
    U_b = np.asarray(U_b, dtype=np.float32)
    V = np.asarray(V, dtype=np.float32)

    if "nc" not in _CACHE:
        _CACHE["nc"] = _build()
    nc = _CACHE["nc"]

    UT = np.ascontiguousarray(U.T)
    WT = np.ascontiguousarray(W.T)
    Ubc = np.ascontiguousarray(U_b.reshape(D, 1))
    Vbc = np.ascontiguousarray(V.reshape(D, 1).astype(ml_dtypes.bfloat16))

    in_maps = []
    for i in range(N_CORES):
        b, h = divmod(i, 2)
        l0 = h * LSH
        in_maps.append(
            {
                "ctxT": np.ascontiguousarray(ctx[b].T),
                "hidT": np.ascontiguousarray(hidden[b, l0 : l0 + LSH, :].T),
                "UT": UT,
                "WT": WT,
                "Ub": Ubc,
                "Vb": Vbc,
            }
        )

    trace = os.environ.get("BASS_KERNEL_TRACE", "0") == "1"
    if trace:
        _install_ntff_hook_shim()
    res = run_bass_kernel_spmd(
        nc,
        in_maps,
        core_ids=list(range(N_CORES)),
        trace=trace,
    )
    _CACHE["last_result"] = res

    outp = np.empty((B, L, C), dtype=np.float32)
    for i in range(N_CORES):
        b, h = divmod(i, 2)
        l0 = h * LSH
        outp[b, l0 : l0 + LSH, :] = res.results[i]["out"]
    return outp


# revision 8
# speedup vs baseline: 1.1952x; 1.1952x over previous
"""Bahdanau additive-attention decoder kernel for one TRN2 chip (8 NeuronCores).

Reference computation (B=4, L=128, C=1024, D=512):
    wq = hidden @ W.T                      # [B, L, D]
    uc = ctx @ U.T + U_b                   # [B, C, D]
    align[b,l,c] = sum_d V[d] * tanh(wq[b,l,d] + uc[b,c,d])
    out = softmax(align, axis=-1)          # [B, L, C]

Sharding: pure data-parallel over the (B x L) grid -- core i owns batch
b = i//2 and L-half h = i%2 (64 query rows), with the full ctx[b].  No
collectives are needed; softmax is over the unsharded C axis.

Per-core device algorithm (layout: d on partitions, c on free axis):
    ucT[d, c] = sum_k U.T[k, d-chunk] . ctxT[k, c]      (PE, fp32)
    wqT[d, l] = sum_k W.T[k, d-chunk] . hiddenT[k, l] + U_b[d]
    for each l:  s = tanh(ucT + wqT[:, l])  via ACT bias-fused tanh (bf16 out)
                 align[l, :] += V_chunk.T @ s  (PE matvec, M=1, accum over 4 chunks)
    softmax over free axis, DMA out.

The host pre-transposes the operands (layout prep is part of sharding);
all arithmetic runs on device.
"""

import os

import numpy as np
import ml_dtypes

import concourse.bass as bass
import concourse.mybir as mybir
import concourse.tile as tile
from concourse import bacc
from concourse.bass_utils import run_bass_kernel_spmd

B, L, C, D = 4, 128, 1024, 512
N_CORES = 8
LSH = (B * L) // N_CORES  # 64 query rows per core
KCH = D // 128  # 4 contraction chunks of 128
F32 = mybir.dt.float32
BF16 = mybir.dt.bfloat16

_CACHE = {}


def _install_ntff_hook_shim():
    """The agent image's antenv lacks axon_hooks; provide it so
    run_bass_kernel_spmd(trace=True) can capture NTFF profiles through
    the axon .so (same mechanism trn_boot would have installed)."""
    import sys
    import types

    try:
        from antenv.axon_hooks import get_axon_ntff_profile_hook

        if get_axon_ntff_profile_hook() is not None:
            return
    except ImportError:
        mod = types.ModuleType("antenv.axon_hooks")
        mod._hook = None

        def set_axon_ntff_profile_hook(h):
            mod._hook = h

        def get_axon_ntff_profile_hook():
            return mod._hook

        mod.set_axon_ntff_profile_hook = set_axon_ntff_profile_hook
        mod.get_axon_ntff_profile_hook = get_axon_ntff_profile_hook
        sys.modules["antenv.axon_hooks"] = mod
        import antenv

        antenv.axon_hooks = mod

    from trn_agent_boot.trn_boot import _ntff_profile_via_ctypes
    import antenv.axon_hooks as ah

    for so in ("/opt/axon/libaxon_pjrt.so",):
        if os.path.exists(so):
            hook = _ntff_profile_via_ctypes(so)
            if hook is not None:
                ah.set_axon_ntff_profile_hook(hook)
                return


def _build():
    nc = bacc.Bacc(
        "TRN2",
        target_bir_lowering=False,
        debug=False,
        num_devices=N_CORES,
    )

    ctxT = nc.dram_tensor("ctxT", (D, C), F32, kind="ExternalInput").ap()
    hidT = nc.dram_tensor("hidT", (D, LSH), F32, kind="ExternalInput").ap()
    UT = nc.dram_tensor("UT", (D, D), F32, kind="ExternalInput").ap()
    WT = nc.dram_tensor("WT", (D, D), F32, kind="ExternalInput").ap()
    Ub = nc.dram_tensor("Ub", (D, 1), F32, kind="ExternalInput").ap()
    Vb = nc.dram_tensor("Vb", (D, 1), BF16, kind="ExternalInput").ap()
    out = nc.dram_tensor("out", (LSH, C), F32, kind="ExternalOutput").ap()

    with tile.TileContext(nc) as tc:
        with (
            tc.tile_pool(name="consts", bufs=1) as cp,
            tc.tile_pool(name="s_pool", bufs=2) as sp,
            tc.tile_pool(name="uc_ps", bufs=2, space="PSUM") as ucp,
            tc.tile_pool(name="al_ps", bufs=2, space="PSUM") as alp,
        ):
            # ---- stage inputs in SBUF ----
            ctxT_t = []
            UT_t = []
            WT_t = []
            hidT_t = []
            ub_t = []
            vb_t = []
            for k in range(KCH):
                sl = slice(k * 128, (k + 1) * 128)
                t = cp.tile([128, C], F32, name=f"ctxT{k}", tag=f"ctxT{k}")
                nc.sync.dma_start(out=t, in_=ctxT[sl, :])
                ctxT_t.append(t)
                t = cp.tile([128, D], F32, name=f"UT{k}", tag=f"UT{k}")
                nc.sync.dma_start(out=t, in_=UT[sl, :])
                UT_t.append(t)
                t = cp.tile([128, D], F32, name=f"WT{k}", tag=f"WT{k}")
                nc.sync.dma_start(out=t, in_=WT[sl, :])
                WT_t.append(t)
                t = cp.tile([128, LSH], F32, name=f"hidT{k}", tag=f"hidT{k}")
                nc.sync.dma_start(out=t, in_=hidT[sl, :])
                hidT_t.append(t)
                t = cp.tile([128, 1], F32, name=f"ub{k}", tag=f"ub{k}")
                nc.sync.dma_start(out=t, in_=Ub[sl, :])
                ub_t.append(t)
                t = cp.tile([128, 1], BF16, name=f"vb{k}", tag=f"vb{k}")
                nc.sync.dma_start(out=t, in_=Vb[sl, :])
                vb_t.append(t)

            # ---- ucT[e_chunk] = (ctx @ U.T).T in [d-part, c-free] layout ----
            ucT_t = []
            for e in range(KCH):
                esl = slice(e * 128, (e + 1) * 128)
                uct = cp.tile([128, C], F32, name=f"ucT{e}", tag=f"ucT{e}")
                for half in range(2):
                    hsl = slice(half * 512, (half + 1) * 512)
                    ps = ucp.tile([128, 512], F32, name=f"ucps{e}_{half}", tag="ucps")
                    for k in range(KCH):
                        nc.tensor.matmul(
                            ps,
                            lhsT=UT_t[k][:, esl],
                            rhs=ctxT_t[k][:, hsl],
                            start=(k == 0),
                            stop=(k == KCH - 1),
                        )
                    nc.vector.tensor_copy(out=uct[:, hsl], in_=ps)
                ucT_t.append(uct)

            # ---- wqT[e_chunk] = (hidden @ W.T).T + U_b, [d-part, l-free] ----
            wqT_t = []
            for e in range(KCH):
                esl = slice(e * 128, (e + 1) * 128)
                ps = ucp.tile([128, LSH], F32, name=f"wqps{e}", tag="wqps")
                for k in range(KCH):
                    nc.tensor.matmul(
                        ps,
                        lhsT=WT_t[k][:, esl],
                        rhs=hidT_t[k],
                        start=(k == 0),
                        stop=(k == KCH - 1),
                    )
                wqt = cp.tile([128, LSH], F32, name=f"wqT{e}", tag=f"wqT{e}")
                nc.vector.tensor_scalar_add(out=wqt, in0=ps, scalar1=ub_t[e])
                wqT_t.append(wqt)

            align_t = cp.tile([LSH, C], F32, name="align", tag="align")

            # ---- main loop: tanh (ACT, bias-fused) + V matvec (PE) ----
            # PE emits align rows as [1, C] on PSUM partition 0; compute
            # engines cannot shift partitions, so rows are staged on
            # partition 0 and scattered into align_t rows by SBUF->SBUF DMA
            # in groups of GRP.
            GRP = 8
            for g in range(LSH // GRP):
                stage = sp.tile([1, GRP, C], F32, name=f"stage{g}", tag="stage")
                for j in range(GRP):
                    l = g * GRP + j
                    s_tiles = []
                    for k in range(KCH):
                        st = sp.tile([128, C], BF16, name=f"s{k}_{l}", tag=f"s{k}")
                        nc.scalar.activation(
                            out=st,
                            in_=ucT_t[k],
                            func=mybir.ActivationFunctionType.Tanh,
                            bias=wqT_t[k][:, l : l + 1],
                            scale=1.0,
                        )
                        s_tiles.append(st)
                    ps = alp.tile([1, C], F32, name=f"alps{l}", tag="al")
                    for half in range(2):
                        hsl = slice(half * 512, (half + 1) * 512)
                        for k in range(KCH):
                            nc.tensor.matmul(
                                ps[0:1, hsl],
                                lhsT=vb_t[k],
                                rhs=s_tiles[k][:, hsl],
                                start=(k == 0),
                                stop=(k == KCH - 1),
                            )
                    nc.vector.tensor_copy(out=stage[0:1, j, :], in_=ps)
                nc.sync.dma_start(
                    out=align_t[g * GRP : (g + 1) * GRP, :], in_=stage[0:1, :, :]
                )

            # ---- softmax over free axis (c) ----
            negmax = cp.tile([LSH, 1], F32, name="negmax", tag="negmax")
            nc.vector.tensor_reduce(
                out=negmax,
                in_=align_t,
                axis=mybir.AxisListType.X,
                op=mybir.AluOpType.max,
                negate=True,
            )
            esum = cp.tile([LSH, 1], F32, name="esum", tag="esum")
            nc.scalar.activation(
                out=align_t,
                in_=align_t,
                func=mybir.ActivationFunctionType.Exp,
                bias=negmax,
                scale=1.0,
                accum_out=esum,
            )
            rec = cp.tile([LSH, 1], F32, name="rec", tag="rec")
            nc.vector.reciprocal(out=rec, in_=esum)
            nc.vector.tensor_scalar_mul(out=align_t, in0=align_t, scalar1=rec)
            nc.sync.dma_start(out=out, in_=align_t)

    nc.compile()
    return nc


def kernel(hidden, ctx, W, U, U_b, V):
    hidden = np.asarray(hidden, dtype=np.float32)
    ctx = np.asarray(ctx, dtype=np.float32)
    W = np.asarray(W, dtype=np.float32)
    U = np.asarray(U, dtype=np.float32)
    U_b = np.asarray(U_b, dtype=np.float32)
    V = np.asarray(V, dtype=np.float32)

    if "nc" not in _CACHE:
        _CACHE["nc"] = _build()
    nc = _CACHE["nc"]

    UT = np.ascontiguousarray(U.T)
    WT = np.ascontiguousarray(W.T)
    Ubc = np.ascontiguousarray(U_b.reshape(D, 1))
    Vbc = np.ascontiguousarray(V.reshape(D, 1).astype(ml_dtypes.bfloat16))

    in_maps = []
    for i in range(N_CORES):
        b, h = divmod(i, 2)
        l0 = h * LSH
        in_maps.append(
            {
                "ctxT": np.ascontiguousarray(ctx[b].T),
                "hidT": np.ascontiguousarray(hidden[b, l0 : l0 + LSH, :].T),
                "UT": UT,
                "WT": WT,
                "Ub": Ubc,
                "Vb": Vbc,
            }
        )

    trace = os.environ.get("BASS_KERNEL_TRACE", "0") == "1"
    if trace:
        _install_ntff_hook_shim()
    res = run_bass_kernel_spmd(
        nc,
        in_maps,
        core_ids=list(range(N_CORES)),
        trace=trace,
    )
    _CACHE["last_result"] = res

    outp = np.empty((B, L, C), dtype=np.float32)
    for i in range(N_CORES):
        b, h = divmod(i, 2)
        l0 = h * LSH
        outp[b, l0 : l0 + LSH, :] = res.results[i]["out"]
    return outp
